# revision 24
# baseline (speedup 1.0000x reference)
"""Location-sensitive attention on 8 Trainium2 NeuronCores.

Data-parallel over batch: B=64 -> 8 batches per core; weights replicated.

Per-core dataflow (per batch, fully pipelined by the Tile scheduler):
  1. DMA value[b] [2048,512] into SBUF native tiles [s128, e] (f32r).
  2. PE-transpose 128x128 blocks -> PSUM -> ACT/DVE copy -> vt tiles [e128, s512].
  3. PE: psum_v[a128, s512] = sum_j Wv_j.T @ vt_j  (+ Weff.T @ R window, the
     conv location term folded via Weff = conv_w.T @ Wloc).
  4. ACT: t = tanh(psum_v + (q@Wq + attn_bias)[a]) using per-partition bias.
  5. PE: energy columns ecol[s128, c] = t_slice.T @ w_score  (fp32, exact).
  6. ACT: exp(ecol) with accum_out -> per-partition partial sums; PE ones-matmul
     reduces partitions; DVE reciprocal; PE K=1 matmul broadcasts 1/sum; DVE
     scales -> align columns [s128, 16].  (max-subtraction dropped: |energy| is
     bounded by ||w_score||_1 ~ 5.7, exp never overflows; b_score cancels in
     softmax and is dropped.)
  7. PE: context[1, 512] accumulates align_col.T @ value_chunk over 16 chunks.
  8. DMA out: context row + align in [s128, c16] layout (host reorders).

Heavy-path dtype is bf16 (PE transposes at 1 cyc/row, full-rate matmuls);
energy/softmax chain stays fp32 (exact N=1 matmuls).  Context accumulates
exp-weighted value and is scaled by 1/sum at the end, so the PE context
matmuls never wait on the normalize chain.  Measured end-to-end relative
error vs the fp32 jax reference: 1.6e-3.  Cost-model (TimelineSim) estimate:
~103 us per core, vs a ~90 us HBM-stream floor for the fp32 value tensor
(32 MB/core at ~360 GB/s).  Half the context contraction is offloaded to
VectorE (per-partition weighted accumulate, folded back by one PE
ones-matmul), relieving the binding TensorE by ~7 us.  Set LSA_HEAVY=f32r for a
~1e-4-accurate variant (~158 us modeled).
"""

import sys

sys.path.insert(0, "/opt/trn_rl_repo")

import numpy as np

import concourse.bass as bass
import concourse.tile as tile
from concourse import mybir
from concourse.bass_utils import run_bass_kernel_spmd

F32 = mybir.dt.float32
F32R = mybir.dt.float32r
BF16 = mybir.dt.bfloat16

N_CORES = 8
B, S, E, RNN, A = 64, 2048, 512, 1024, 128
NB = B // N_CORES          # batches per core
NS = S // 128              # 16 s-blocks of 128
NE = E // 128              # 4 e-chunks
NC_S = S // 512            # 4 s-chunks of 512
APW = 2112                 # padded alignment row (15 | 2048 | 15 | slack)
WINW = S + 30              # 2078 window width

import os
HEAVY = os.environ.get("LSA_HEAVY", "bf16")  # "f32r" (~1.5e-4) or "bf16" (~5e-3)
VALUE_HBM = os.environ.get("LSA_VALUE_HBM", "f32")  # "f32" or "bf16" (host-cast)


def _split_multi_waits(nc, max_waits=1):
    """This walrus build rejects >1 sync-wait per instruction; hoist extras
    onto preceding same-engine NoOps (program order preserves semantics)."""
    cnt = 0
    for f in nc.m.functions:
        for bb in f.blocks:
            new = []
            for inst in bb.instructions:
                si = getattr(inst, "sync_info", None)
                if si is not None and si.on_wait and len(si.on_wait) > max_waits:
                    waits = list(si.on_wait)
                    si.on_wait = waits[-max_waits:]
                    head = waits[:-max_waits]
                    for i in range(0, len(head), max_waits):
                        n = mybir.InstNoOp(name=f"SPLITW-{cnt}", ins=[], outs=[])
                        cnt += 1
                        n.engine = inst.engine
                        n.sync_info = mybir.SyncInfo(
                            on_wait=head[i : i + max_waits], on_update=[]
                        )
                        new.append(n)
                new.append(inst)
            bb.instructions[:] = new
    return cnt


def build_module(heavy=HEAVY, vbufs=4, vtbufs=12, vtpbufs=4, pvbufs=1, pecbufs=1, dve_copies=8, r_merge=False, tbufs=3, ablate=(), value_hbm=None, pcxbufs=1, psmbufs=1, smbufs=5, vsplit=4, warmup=0, ctx_dve=8, exp_split=False):
    value_hbm = VALUE_HBM if value_hbm is None else value_hbm
    HV = F32R if heavy == "f32r" else BF16
    nc = bass.Bass("TRN2", target_bir_lowering=False, debug=False,
                   num_devices=N_CORES)

    def hv_load(dst_ap, src_ap):
        # Load an fp32 DRAM region into an HV-typed SBUF tile.
        if HV is F32R:
            nc.sync.dma_start(dst_ap, src_ap.bitcast(F32R))
        else:
            nc.gpsimd.dma_start(dst_ap, src_ap)  # SWDGE casts fp32->bf16

    VDT = BF16 if (value_hbm == "bf16" and heavy == "bf16") else F32
    value_t = nc.dram_tensor("value", [NB, S, E], VDT, kind="ExternalInput")
    apad_t = nc.dram_tensor("apad", [NB, APW], F32, kind="ExternalInput")
    query_t = nc.dram_tensor("query", [NB, RNN], F32, kind="ExternalInput")
    wq_t = nc.dram_tensor("wq", [RNN, A], F32, kind="ExternalInput")
    wv_t = nc.dram_tensor("wv", [E, A], F32, kind="ExternalInput")
    wloc_t = nc.dram_tensor("wloc", [32, A], F32, kind="ExternalInput")
    convw_t = nc.dram_tensor("convw", [32, 31], F32, kind="ExternalInput")
    abias_t = nc.dram_tensor("abias", [A], F32, kind="ExternalInput")
    wsc_t = nc.dram_tensor("wsc", [A, 1], F32, kind="ExternalInput")
    ident_t = nc.dram_tensor("ident", [128, 128], F32, kind="ExternalInput")
    onesv_t = nc.dram_tensor("onesv", [128], F32, kind="ExternalInput")

    ctx_t = nc.dram_tensor("ctx", [NB, E], F32, kind="ExternalOutput")
    alT_t = nc.dram_tensor("alT", [NB, 128, NS], F32, kind="ExternalOutput")

    AF = mybir.ActivationFunctionType

    with tile.TileContext(nc) as tc:
        import contextlib

        with contextlib.ExitStack() as ctx:
            cst = ctx.enter_context(tc.tile_pool(name="cst", bufs=1))

            ident_hv = cst.tile([128, 128], HV)
            hv_load(ident_hv[:], ident_t.ap())
            ident8 = cst.tile([8, 8], F32)
            nc.sync.dma_start(ident8[:], ident_t.ap()[0:8, 0:8])
            ones_col = cst.tile([128, 1], F32)
            nc.sync.dma_start(ones_col[:], bass.AP(onesv_t, 0, [[1, 128], [1, 1]]))
            ones_row = cst.tile([1, 128], F32)
            nc.sync.dma_start(ones_row[:], bass.AP(onesv_t, 0, [[1, 1], [1, 128]]))
            onesb_col = cst.tile([128, 1], HV)
            hv_load(onesb_col[:], bass.AP(onesv_t, 0, [[1, 128], [1, 1]]))
            wv_sb = cst.tile([128, NE * 128], HV)
            hv_load(wv_sb[:], bass.AP(wv_t, 0, [[A, 128], [128 * A, NE], [1, A]]))
            wq_sb = cst.tile([128, 8 * 128], F32)
            nc.sync.dma_start(
                wq_sb[:], bass.AP(wq_t, 0, [[A, 128], [128 * A, 8], [1, A]])
            )
            wloc_sb = cst.tile([32, A], F32)
            nc.sync.dma_start(wloc_sb[:], wloc_t.ap())
            convw_sb = cst.tile([32, 31], F32)
            nc.sync.dma_start(convw_sb[:], convw_t.ap())
            bias_sb = cst.tile([128, 1], F32)
            nc.sync.dma_start(bias_sb[:], bass.AP(abias_t, 0, [[1, 128], [1, 1]]))
            wsc_sb = cst.tile([128, 1], F32)
            nc.sync.dma_start(wsc_sb[:], wsc_t.ap())
            q_nat = cst.tile([8, RNN], F32)
            nc.sync.dma_start(q_nat[:], query_t.ap())
            weff_sb = cst.tile([31, A], HV)
            qb_sb = cst.tile([128, NB], F32)

            # ---- phase 0: Weff = conv_w.T @ Wloc ; qb = query @ Wq + bias
            with tc.tile_pool(name="ps0", bufs=1, space="PSUM") as ps0:
                pw = ps0.tile([31, A], F32)
                nc.tensor.matmul(pw[:], convw_sb[:], wloc_sb[:], start=True, stop=True)
                with nc.allow_low_precision(reason="feeds f32r PE path"):
                    nc.scalar.copy(weff_sb[:], pw[:])

                qt_ps = ps0.tile([128, 64], F32)
                for j in range(8):
                    nc.tensor.transpose(
                        qt_ps[:, j * 8 : (j + 1) * 8],
                        q_nat[:, j * 128 : (j + 1) * 128],
                        ident8[:],
                    )
                qt_sb = cst.tile([128, 64], F32)
                nc.scalar.copy(qt_sb[:], qt_ps[:])
                pq = ps0.tile([128, NB], F32)
                for j in range(8):
                    nc.tensor.matmul(
                        pq[:],
                        wq_sb[:, j * 128 : (j + 1) * 128],
                        qt_sb[:, j * 8 : (j + 1) * 8],
                        start=(j == 0),
                        stop=(j == 7),
                    )
                nc.vector.tensor_scalar_add(qb_sb[:], pq[:], bias_sb[:])

            # ---- PE warmup: keep TensorE busy during the first value DMA so
            # the HAM clock gate is at 8/8 (2.4 GHz) when real work arrives.
            if warmup:
                with tc.tile_pool(name="pwu", bufs=1, space="PSUM") as pwu:
                    wu_ps = pwu.tile([128, 128], HV)
                    for _ in range(warmup):
                        nc.tensor.transpose(wu_ps[:], ident_hv[:], ident_hv[:])

            # ---- phase 1: per-batch pipeline
            vpool = ctx.enter_context(tc.tile_pool(name="vnat", bufs=vbufs))
            rpool = ctx.enter_context(tc.tile_pool(name="rwin", bufs=2))
            vtpool = ctx.enter_context(tc.tile_pool(name="vt", bufs=vtbufs))
            tpool = ctx.enter_context(tc.tile_pool(name="tt", bufs=tbufs))
            sm = ctx.enter_context(tc.tile_pool(name="sm", bufs=smbufs))
            vtp = ctx.enter_context(tc.tile_pool(name="vtp", bufs=vtpbufs, space="PSUM"))
            pvp = ctx.enter_context(tc.tile_pool(name="pvp", bufs=pvbufs, space="PSUM"))
            pec = ctx.enter_context(tc.tile_pool(name="pec", bufs=pecbufs, space="PSUM"))
            psm = ctx.enter_context(tc.tile_pool(name="psm", bufs=psmbufs, space="PSUM"))
            pcx = ctx.enter_context(tc.tile_pool(name="pcx", bufs=pcxbufs, space="PSUM"))

            r_all = None
            if r_merge:
                r_all = cst.tile([31, NB * WINW], HV)
                hv_load(r_all[:], bass.AP(apad_t, 0, [[1, 31], [APW, NB], [1, WINW]]))
            for b in range(NB):
                v_nat = vpool.tile([128, NS * E], HV)
                nsp = NS // vsplit
                for h in range(vsplit):
                    v_src = bass.AP(
                        value_t,
                        b * S * E + h * nsp * 128 * E,
                        [[E, 128], [128 * E, nsp], [1, E]],
                    )
                    dst = v_nat[:, h * nsp * E : (h + 1) * nsp * E]
                    if VDT is BF16:
                        nc.sync.dma_start(dst, v_src)
                    else:
                        hv_load(dst, v_src)
                if r_merge:
                    r_sb = r_all[:, b * WINW : (b + 1) * WINW]
                else:
                    r_sb = rpool.tile([31, WINW], HV)
                    hv_load(r_sb[:], bass.AP(apad_t, b * APW, [[1, 31], [1, WINW]]))

                ecol = pec.tile([128, NS], F32)
                copy_alt = 0
                for C in range(NC_S):
                    vts = []
                    for j in range(NE):
                        if "tpcopy" in ablate:
                            vts.append(None)
                            continue
                        if "tp" in ablate:
                            vt_ps = vtp.tile([128, 512], HV)
                            nc.gpsimd.memset(vt_ps[:], 0)
                        else:
                            vt_ps = vtp.tile([128, 512], HV)
                            for i in range(4):
                                c = 4 * C + i
                                nc.tensor.transpose(
                                    vt_ps[:, i * 128 : (i + 1) * 128],
                                    v_nat[:, c * E + j * 128 : c * E + (j + 1) * 128],
                                    ident_hv[:],
                                )
                        vt_sb = vtpool.tile([128, 512], HV)
                        if (copy_alt * dve_copies) % 16 < dve_copies:
                            with nc.allow_low_precision(reason="f32r copy"):
                                nc.vector.tensor_copy(vt_sb[:], vt_ps[:])
                        else:
                            nc.scalar.copy(vt_sb[:], vt_ps[:])
                        copy_alt += 1
                        vts.append(vt_sb)

                    pv = pvp.tile([128, 512], F32)
                    for j in range(NE):
                        rhs = vts[j][:] if vts[j] is not None else wv_sb[:, 0:512]
                        nc.tensor.matmul(
                            pv[:],
                            wv_sb[:, j * 128 : (j + 1) * 128],
                            rhs,
                            start=(j == 0),
                            stop=False,
                        )
                    nc.tensor.matmul(
                        pv[:],
                        weff_sb[:],
                        r_sb[:, C * 512 : C * 512 + 512],
                        start=False,
                        stop=True,
                    )
                    t_sb = tpool.tile([128, 512], F32)
                    nc.scalar.activation(
                        t_sb[:], pv[:], AF.Tanh, bias=qb_sb[:, b : b + 1]
                    )
                    for i in range(4):
                        c = 4 * C + i
                        nc.tensor.matmul(
                            ecol[:, c : c + 1],
                            t_sb[:, i * 128 : (i + 1) * 128],
                            wsc_sb[:],
                            start=True,
                            stop=True,
                        )

                # softmax over all S for this batch (no max-sub needed; bounded)
                expcol = sm.tile([128, NS], F32)
                if exp_split:
                    partial = sm.tile([128, 4], F32)
                    for g in range(4):
                        nc.scalar.activation(
                            expcol[:, g * 4 : (g + 1) * 4],
                            ecol[:, g * 4 : (g + 1) * 4],
                            AF.Exp, accum_out=partial[:, g : g + 1],
                        )
                    s4 = psm.tile([4, 1], F32, tag="small")
                    nc.tensor.matmul(s4[:], partial[:], ones_col[:], start=True, stop=True)
                    s4s = sm.tile([4, 1], F32)
                    nc.vector.tensor_copy(s4s[:], s4[:])
                    sump = psm.tile([1, 1], F32, tag="small")
                    nc.tensor.matmul(sump[:], s4s[:], ones_col[0:4, :], start=True, stop=True)
                else:
                    partial = sm.tile([128, 1], F32)
                    nc.scalar.activation(
                        expcol[:], ecol[:], AF.Exp, accum_out=partial[:]
                    )
                    sump = psm.tile([1, 1], F32, tag="small")
                    nc.tensor.matmul(sump[:], partial[:], ones_col[:], start=True, stop=True)
                recip = sm.tile([1, 1], F32)
                nc.vector.reciprocal(recip[:], sump[:])
                rb = psm.tile([128, 1], F32, tag="small")
                nc.tensor.matmul(rb[:], ones_row[:], recip[:], start=True, stop=True)
                # context accumulates exp-weighted value; normalized at the
                # end by 1/sum so it never waits on the normalize chain.
                acast = sm.tile([128, NS], HV)
                with nc.allow_low_precision(reason="feeds PE"):
                    nc.vector.tensor_copy(acast[:], expcol[:])
                # context: last ctx_dve chunks pre-reduced on DVE (weighted
                # accumulate across partitions' free dim), folded into the PE
                # accumulation group by one ones-matmul; rest stay on PE.
                n_pe = NS - ctx_dve
                acc = None
                if ctx_dve:
                    acc = sm.tile([128, E], HV, tag="ctxacc")
                    nc.vector.tensor_scalar_mul(
                        acc[:], v_nat[:, n_pe * E : (n_pe + 1) * E],
                        expcol[:, n_pe : n_pe + 1])
                    for c in range(n_pe + 1, NS):
                        tmp = sm.tile([128, E], HV, tag="ctxtmp")
                        nc.vector.tensor_scalar_mul(
                            tmp[:], v_nat[:, c * E : (c + 1) * E],
                            expcol[:, c : c + 1])
                        nc.vector.tensor_tensor(
                            out=acc[:], in0=acc[:], in1=tmp[:],
                            op=mybir.AluOpType.add)
                pctx = pcx.tile([1, E], F32)
                nctx = 1 if "ctx" in ablate else n_pe
                for c in range(nctx):
                    nc.tensor.matmul(
                        pctx[:],
                        acast[:, c : c + 1],
                        v_nat[:, c * E : (c + 1) * E],
                        start=(c == 0),
                        stop=(c == nctx - 1) and not ctx_dve,
                    )
                if ctx_dve:
                    nc.tensor.matmul(
                        pctx[:], onesb_col[:], acc[:], start=False, stop=True)
                rbs = sm.tile([128, 1], F32)
                nc.vector.tensor_copy(rbs[:], rb[:])
                align_sb = sm.tile([128, NS], F32)
                nc.vector.tensor_scalar_mul(align_sb[:], expcol[:], rbs[:])
                nc.sync.dma_start(
                    bass.AP(alT_t, b * 128 * NS, [[NS, 128], [1, NS]]), align_sb[:]
                )
                ctx_sb = sm.tile([1, E], F32)
                nc.vector.tensor_scalar_mul(ctx_sb[:], pctx[:], recip[:])
                nc.sync.dma_start(
                    bass.AP(ctx_t, b * E, [[1, 1], [1, E]]), ctx_sb[:]
                )

    _split_multi_waits(nc)
    return nc


_NC_CACHE = {}


def _get_module(heavy=HEAVY):
    if heavy not in _NC_CACHE:
        _NC_CACHE[heavy] = build_module(heavy)
    return _NC_CACHE[heavy]


def make_in_maps(query, value, last_alignment_energy, Wq, Wv, conv_w, Wloc,
                 attn_bias, w_score):
    query = np.asarray(query, np.float32).reshape(B, RNN)
    value = np.asarray(value, np.float32)
    if VALUE_HBM == "bf16" and HEAVY == "bf16":
        import ml_dtypes
        value = value.astype(ml_dtypes.bfloat16)
    lae = np.asarray(last_alignment_energy, np.float32)
    apad = np.zeros((B, APW), np.float32)
    apad[:, 15 : 15 + S] = lae
    shared = {
        "wq": np.asarray(Wq, np.float32),
        "wv": np.asarray(Wv, np.float32),
        "wloc": np.asarray(Wloc, np.float32),
        "convw": np.ascontiguousarray(np.asarray(conv_w, np.float32)[:, 0, :]),
        "abias": np.asarray(attn_bias, np.float32),
        "wsc": np.asarray(w_score, np.float32),
        "ident": np.eye(128, dtype=np.float32),
        "onesv": np.ones(128, np.float32),
    }
    in_maps = []
    for i in range(N_CORES):
        lo = i * NB
        in_maps.append(
            {
                "value": np.ascontiguousarray(value[lo : lo + NB]),
                "apad": np.ascontiguousarray(apad[lo : lo + NB]),
                "query": np.ascontiguousarray(query[lo : lo + NB]),
                **shared,
            }
        )
    return in_maps


def assemble(results):
    ctx = np.concatenate([results[i]["ctx"] for i in range(N_CORES)], axis=0)
    alT = np.concatenate([results[i]["alT"] for i in range(N_CORES)], axis=0)
    align = np.ascontiguousarray(alT.transpose(0, 2, 1)).reshape(B, S)
    return ctx, align


def kernel(query, value, last_alignment_energy, Wq, Wv, conv_w, Wloc,
           attn_bias, w_score, b_score):
    # b_score drops out of both outputs (softmax shift-invariance).
    nc = _get_module()
    in_maps = make_in_maps(query, value, last_alignment_energy, Wq, Wv, conv_w,
                           Wloc, attn_bias, w_score)
    res = run_bass_kernel_spmd(nc, in_maps, core_ids=list(range(N_CORES)))
    return assemble(res.results)


if __name__ == "__main__":
    nc = build_module()
    print("module built ok; instructions:",
          sum(len(bb.instructions) for f in nc.m.functions for bb in f.blocks))
